# revision 21
# baseline (speedup 1.0000x reference)
"""Trainium2 Bass kernel for nn_MixtureOfExperts_30210799960341.

Expert-parallel MoE over 8 NeuronCores (one expert per core):
  - each core computes the router (linear -> softmax -> argmax) for its
    1/8 slice of the tokens (fp32 matmul, argmax-stable),
  - routes are AllGathered so every core sees all 8192 routing decisions,
  - each core stream-compacts (gpsimd sparse_gather) the token ids routed
    to its expert, gathers those rows of x straight from HBM with an
    indexed DMA (dma_gather), runs its expert's 2-layer MLP on them
    (fp32r matmuls on the PE array), and writes back compacted rows,
  - host scatters the compacted rows into the full [N, O] output using the
    device-computed index lists (the "combine" / unshard step).

kernel(**inputs) takes the FULL unsharded inputs and returns the FULL
outputs (logits [8192,512], router_probs [8192,8], counts [8]).
"""

import os
import sys

for _p in ("/opt/trn_rl_repo", "/root/.axon_site/_ro/trn_rl_repo"):
    if _p not in sys.path:
        sys.path.insert(0, _p)

import numpy as np

import concourse.bass as bass
import concourse.mybir as mybir
import concourse.tile as tile
from concourse import bacc, bass_utils, library_config
from concourse.tile import add_dep_helper
from concourse.bass_interp import get_hw_module

# Problem shapes (hardcoded per the spec).
N, D, H, O, E = 8192, 512, 2048, 512, 8
NC = 8              # cores
NLOC = N // NC      # tokens routed per core slice = 1024
TLOC = NLOC // 128  # 128-token tiles per slice = 8
CAP = int(os.environ.get("K_CAP", "1536"))  # per-expert token capacity
CTILES = CAP // 128  # 12
GROUPS = CTILES // 4  # 3 groups of 512 tokens
SG_F = 512          # sparse_gather output free dim (16*512 slots, never overflows)
IDX_F = CAP // 16   # 96
KD = D // 128       # 4
HT = H // 128       # 16

F32 = mybir.dt.float32
F32R = mybir.dt.float32r
I16 = mybir.dt.int16
U32 = mybir.dt.uint32
ALU = mybir.AluOpType
ACTF = mybir.ActivationFunctionType

_CACHE = {}
PHASE = int(os.environ.get("K_PHASE", "9"))


def _emit(nc, tc, ten):
    ctx_frees = []

    def persist(shape, dtype, name):
        t, free = tc.tile(shape, dtype, name=name)
        ctx_frees.append(free)
        return t

    # ---- persistent SBUF tensors ----------------------------------------
    w1_sb = persist([128, KD, H], F32R, "w1_sb")        # 32KB/part
    w2_sb = persist([128, HT, O], F32R, "w2_sb")        # 32KB/part
    wr_sb = persist([128, KD, E], F32, "wr_sb")
    br_sb = persist([128, E], F32, "br_sb")
    b1t_sb = persist([128, HT], F32, "b1t_sb")
    b2_sb = persist([128, O], F32, "b2_sb")
    id_sb = persist([128, 128], F32, "id_sb")
    iota_sb = persist([16, SG_F], F32, "iota_sb")
    iotae_sb = persist([128, TLOC * E], F32, "iotae_sb")
    eid_sb = persist([16, 1], F32, "eid_sb")
    rep16_sb = persist([16, 128], F32, "rep16_sb")
    idxf_sb = persist([128, IDX_F], F32, "idxf_sb")
    zeros_sb = persist([128, O], F32, "zeros_sb")
    routes8 = persist([128, TLOC], F32, "routes8")
    l_all = persist([128, TLOC * E], F32, "l_all")
    xg = persist([128, CTILES, D], F32, "xg")          # 24KB/part
    idx_rep = persist([128, IDX_F], I16, "idx_rep")
    nf_sb = persist([1, 1], U32, "nf_sb")
    sg_sb = persist([16, IDX_F], F32, "sg_sb")
    v_sb = persist([16, 1 + SG_F], F32, "v_sb")
    routes16 = persist([16, SG_F], F32, "routes16")

    nc.sync.dma_start(w1_sb[:], ten["w1"].ap().rearrange("(k p) h -> p k h", p=128))
    nc.sync.dma_start(w2_sb[:], ten["w2"].ap().rearrange("(k p) o -> p k o", p=128))
    nc.sync.dma_start(wr_sb[:], ten["wr"].ap().rearrange("(k p) e -> p k e", p=128))
    nc.sync.dma_start(b1t_sb[:], ten["b1t"].ap())
    nc.sync.dma_start(id_sb[:], ten["id128"].ap())
    nc.sync.dma_start(iota_sb[:], ten["iota1"].ap())
    nc.sync.dma_start(iotae_sb[:], ten["iotaE"].ap())
    nc.sync.dma_start(eid_sb[:], ten["eid"].ap())
    nc.sync.dma_start(rep16_sb[:], ten["rep16"].ap())
    nc.vector.memset(zeros_sb[:], 0.0)
    nc.sync.dma_start(br_sb[:], ten["br"].ap())
    nc.sync.dma_start(b2_sb[:], ten["b2"].ap())

    xs_d = ten["xs"]
    x_d = ten["x"]
    probs_d = ten["probs"]
    y_d = ten["y"]
    idx_d = ten["idx"]
    cnt_d = ten["cnt"]

    with tc.tile_pool(name="xs_pool", bufs=3) as xs_pool, \
         tc.tile_pool(name="xt_pool", bufs=2) as xt_pool, \
         tc.tile_pool(name="ht_pool", bufs=1) as ht_pool, \
         tc.tile_pool(name="y_pool", bufs=3) as y_pool, \
         tc.tile_pool(name="small", bufs=1) as small, \
         tc.tile_pool(name="ps_tr", bufs=2, space="PSUM") as ps_tr_pool, \
         tc.tile_pool(name="ps_l", bufs=2, space="PSUM") as ps_l_pool, \
         tc.tile_pool(name="ps_h", bufs=2, space="PSUM") as ps_h_pool, \
         tc.tile_pool(name="ps_y", bufs=2, space="PSUM") as ps_y_pool, \
         tc.tile_pool(name="dram", bufs=1, space="DRAM") as dram:

        # ================= Phase B: router on the local token slice ======
        for t in range(TLOC):
            xs_t = xs_pool.tile([128, D], F32, tag="xs_t")
            nc.sync.dma_start(xs_t[:], xs_d[t * 128:(t + 1) * 128, :])
            ps_tr = ps_tr_pool.tile([128, D], F32, tag="ps_tr")
            for k in range(KD):
                nc.tensor.transpose(ps_tr[:, k * 128:(k + 1) * 128],
                                    xs_t[:, k * 128:(k + 1) * 128], id_sb[:])
            xt_t = xt_pool.tile([128, D], F32, tag="xt_t")
            nc.scalar.copy(xt_t[:], ps_tr[:])
            ps_l = ps_l_pool.tile([128, E], F32, tag="ps_l")
            for k in range(KD):
                nc.tensor.matmul(ps_l[:], xt_t[:, k * 128:(k + 1) * 128],
                                 wr_sb[:, k, :], start=(k == 0),
                                 stop=(k == KD - 1))
            # logits (+ router bias) into the batched [128, (t e)] buffer
            nc.vector.tensor_add(l_all[:, t * E:(t + 1) * E], ps_l[:], br_sb[:, 0:E])

        # ---- batched softmax / argmax over all 8 tiles at once -----------
        l3 = l_all[:].rearrange("p (t e) -> p t e", e=E)
        mx = small.tile([128, TLOC], F32, tag="mx")
        nc.vector.reduce_max(mx[:], l3, axis=mybir.AxisListType.X)
        mx3 = mx[:].unsqueeze(2)
        sub = small.tile([128, TLOC * E], F32, tag="sub")
        a_ap, b_ap = bass.broadcast_tensor_aps(l3, mx3)
        nc.vector.tensor_tensor(sub[:].rearrange("p (t e) -> p t e", e=E),
                                a_ap, b_ap, op=ALU.subtract)
        eall = small.tile([128, TLOC * E], F32, tag="eall")
        nc.scalar.activation(eall[:], sub[:], ACTF.Exp)
        ssum = small.tile([128, TLOC], F32, tag="ssum")
        nc.vector.reduce_sum(ssum[:], eall[:].rearrange("p (t e) -> p t e", e=E),
                             axis=mybir.AxisListType.X)
        rec = small.tile([128, TLOC], F32, tag="rec")
        nc.vector.reciprocal(rec[:], ssum[:])
        probs_sb = small.tile([128, TLOC * E], F32, tag="probs_sb")
        pa, pb = bass.broadcast_tensor_aps(
            eall[:].rearrange("p (t e) -> p t e", e=E),
            rec[:].unsqueeze(2))
        nc.vector.tensor_tensor(probs_sb[:].rearrange("p (t e) -> p t e", e=E),
                                pa, pb, op=ALU.mult)
        nc.sync.dma_start(probs_d.ap().rearrange("t p e -> p t e"),
                          probs_sb[:].rearrange("p (t e) -> p t e", e=E))
        # routes[p, t] = sum_e e * (l == max)
        oh = small.tile([128, TLOC * E], F32, tag="oh")
        oa, ob = bass.broadcast_tensor_aps(l3, mx3)
        nc.vector.tensor_tensor(oh[:].rearrange("p (t e) -> p t e", e=E),
                                oa, ob, op=ALU.is_equal)
        nc.vector.tensor_mul(oh[:], oh[:], iotae_sb[:])
        nc.vector.reduce_sum(routes8[:], oh[:].rearrange("p (t e) -> p t e", e=E),
                             axis=mybir.AxisListType.X)

        # ================= AllGather of routing decisions ================
        if PHASE < 2:
            return
        cc_in = dram.tile([NLOC], F32, name="cc_in")
        cc_out = dram.tile([N], F32, name="cc_out", addr_space="Shared")
        nc.sync.dma_start(cc_in[:].rearrange("(p t) -> p t", p=128), routes8[:])
        nc.gpsimd.collective_compute(
            "AllGather", ALU.bypass,
            replica_groups=[list(range(NC))],
            ins=[cc_in[:]], outs=[cc_out[:]],
        )
        nc.sync.dma_start(routes16[:], cc_out[:].rearrange("(a f) -> a f", a=16))

        # ================= compaction: token ids for my expert ===========
        if PHASE < 3:
            return
        # v[0, 0] is a sentinel (token 0) so the compacted count is >= 1.
        nc.vector.memset(v_sb[:, 0:1], -1.0)
        nc.vector.memset(v_sb[0:1, 0:1], 0.0)
        nc.vector.scalar_tensor_tensor(v_sb[:, 1:1 + SG_F], routes16[:],
                                       eid_sb[:], iota_sb[:],
                                       op0=ALU.is_equal, op1=ALU.mult)
        nc.vector.tensor_scalar_add(v_sb[:, 1:1 + SG_F], v_sb[:, 1:1 + SG_F], -1.0)
        if PHASE < 4:
            return
        lib_sg = nc.gpsimd.load_library(library_config.sparse_gather)
        sg_inst = nc.gpsimd.sparse_gather(sg_sb[:], v_sb[:], num_found=nf_sb[:])
        add_dep_helper(sg_inst.ins, lib_sg.ins, sync=True,
                       reason="sparse_gather needs its ucode library")
        nc.sync.dma_start(idx_d.ap(), sg_sb[:])
        nc.sync.dma_start(cnt_d.ap(), nf_sb[:])
        if PHASE < 5:
            return
        if PHASE < 6:
            return
        ps_rep = ps_l_pool.tile([128, IDX_F], F32, tag="ps_l", name="ps_rep")
        nc.tensor.matmul(ps_rep[:], rep16_sb[:], sg_sb[:], start=True, stop=True)
        if PHASE < 7:
            return
        # relu clamps the -1 padding to token 0: every slot becomes a valid
        # (discarded-on-host) gather index, so the gathers use static counts.
        nc.scalar.activation(idxf_sb[:], ps_rep[:], ACTF.Relu)
        nc.vector.tensor_copy(idx_rep[:], idxf_sb[:])

        # ================= gather routed token rows from full x ==========
        if PHASE < 8:
            return
        lib_mlp = nc.gpsimd.load_library(library_config.mlp)
        add_dep_helper(lib_mlp.ins, sg_inst.ins, sync=True,
                       reason="library switch after sparse_gather ran")
        for gg in range(GROUPS):
            gather_inst = nc.gpsimd.dma_gather(
                out_ap=xg[:, gg * 4:(gg + 1) * 4, :],
                in_ap=x_d.ap(),
                idxs_ap=idx_rep[:, gg * 32:(gg + 1) * 32],
                num_idxs=512,
                num_idxs_reg=512,
                elem_size=D,
            )
            add_dep_helper(gather_inst.ins, lib_mlp.ins, sync=True,
                           reason="dma_gather needs the mlp ucode library")

        # ================= expert MLP on gathered tokens =================
        if PHASE < 9:
            return
        for g in range(GROUPS):
            xt_g = [xt_pool.tile([128, 512], F32R, tag=f"xt_g{k}", name=f"xt_g{k}")
                    for k in range(KD)]
            for k in range(KD):
                ps_tr = ps_tr_pool.tile([128, 512], F32, tag="ps_tr")
                for tb in range(4):
                    nc.tensor.transpose(
                        ps_tr[:, tb * 128:(tb + 1) * 128],
                        xg[:, g * 4 + tb, k * 128:(k + 1) * 128], id_sb[:])
                nc.scalar.copy(xt_g[k][:], ps_tr[:])

            hts = []
            for ht in range(HT):
                ps_h = ps_h_pool.tile([128, 512], F32, tag="ps_h")
                for k in range(KD):
                    nc.tensor.matmul(
                        ps_h[:],
                        w1_sb[:, k, ht * 128:(ht + 1) * 128],
                        xt_g[k][:],
                        start=(k == 0), stop=(k == KD - 1))
                hT = ht_pool.tile([128, 512], F32R, tag=f"hT{ht}", name=f"hT{ht}")
                if ht % 2 == 0:
                    nc.scalar.activation(hT[:], ps_h[:], ACTF.Relu,
                                         bias=b1t_sb[:, ht:ht + 1], scale=1.0)
                else:
                    nc.vector.scalar_tensor_tensor(hT[:], ps_h[:],
                                                   b1t_sb[:, ht:ht + 1],
                                                   zeros_sb[:],
                                                   op0=ALU.add, op1=ALU.max)
                hts.append(hT)

            for tb in range(4):
                ps_y = ps_y_pool.tile([128, O], F32, tag="ps_y")
                for ht in range(HT):
                    nc.tensor.matmul(
                        ps_y[:],
                        hts[ht][:, tb * 128:(tb + 1) * 128],
                        w2_sb[:, ht, :],
                        start=(ht == 0), stop=(ht == HT - 1))
                y_sb = y_pool.tile([128, O], F32, tag="y_sb")
                nc.vector.tensor_add(y_sb[:], ps_y[:], b2_sb[:])
                nc.sync.dma_start(y_d[g * 4 + tb, :, :], y_sb[:])

    for free in reversed(ctx_frees):
        free()


def build_program():
    """Build (once) and return the compiled SPMD Bass program."""
    if "nc" in _CACHE:
        return _CACHE["nc"]
    nc = bacc.Bacc("TRN2", target_bir_lowering=False, debug=False,
                   num_devices=NC)
    ten = {
        "x": nc.dram_tensor("x", [N, D], F32, kind="ExternalInput"),
        "xs": nc.dram_tensor("xs", [NLOC, D], F32, kind="ExternalInput"),
        "wr": nc.dram_tensor("wr", [D, E], F32, kind="ExternalInput"),
        "br": nc.dram_tensor("br", [128, E], F32, kind="ExternalInput"),
        "w1": nc.dram_tensor("w1", [D, H], F32R, kind="ExternalInput"),
        "b1t": nc.dram_tensor("b1t", [128, HT], F32, kind="ExternalInput"),
        "w2": nc.dram_tensor("w2", [H, O], F32R, kind="ExternalInput"),
        "b2": nc.dram_tensor("b2", [128, O], F32, kind="ExternalInput"),
        "id128": nc.dram_tensor("id128", [128, 128], F32, kind="ExternalInput"),
        "iota1": nc.dram_tensor("iota1", [16, SG_F], F32, kind="ExternalInput"),
        "iotaE": nc.dram_tensor("iotaE", [128, TLOC * E], F32, kind="ExternalInput"),
        "eid": nc.dram_tensor("eid", [16, 1], F32, kind="ExternalInput"),
        "rep16": nc.dram_tensor("rep16", [16, 128], F32, kind="ExternalInput"),
        "probs": nc.dram_tensor("probs", [TLOC, 128, E], F32, kind="ExternalOutput"),
        "y": nc.dram_tensor("y", [CTILES, 128, O], F32, kind="ExternalOutput"),
        "idx": nc.dram_tensor("idx", [16, IDX_F], F32, kind="ExternalOutput"),
        "cnt": nc.dram_tensor("cnt", [1, 1], U32, kind="ExternalOutput"),
    }
    with tile.TileContext(nc) as tc:
        _emit(nc, tc, ten)
    nc.compile()
    nc.m = get_hw_module(nc.m)
    _CACHE["nc"] = nc
    return nc


def host_tables():
    """Constant tables shipped to every core."""
    if "iota1" in _CACHE:
        return _CACHE["iota1"], _CACHE["iotaE"]
    # iota1[a, f] = 1 + the global token id whose route sits at wrapped
    # slot (a, f) of the AllGathered routes buffer.
    g = np.arange(16)[:, None] * SG_F + np.arange(SG_F)[None, :]
    c = g // NLOC
    j = g % NLOC
    p = j // TLOC
    t = j % TLOC
    tok = c * NLOC + t * 128 + p
    iota1 = (tok + 1).astype(np.float32)
    iotae = np.tile(np.arange(E, dtype=np.float32), (128, TLOC))
    _CACHE["iota1"] = iota1
    _CACHE["iotaE"] = iotae
    return iota1, iotae


def make_in_maps(x, Wr, br, W1, b1, W2, b2):
    iota1, iotae = host_tables()
    id128 = np.eye(128, dtype=np.float32)
    rep16 = (np.arange(128)[None, :] % 16 == np.arange(16)[:, None]).astype(np.float32)
    in_maps = []
    for c in range(NC):
        in_maps.append({
            "x": x,
            "xs": x[c * NLOC:(c + 1) * NLOC],
            "wr": Wr,
            "br": np.tile(br.reshape(1, E), (128, 1)),
            "w1": W1[c],
            "b1t": np.ascontiguousarray(b1[c].reshape(HT, 128).T),
            "w2": W2[c],
            "b2": np.tile(b2[c].reshape(1, O), (128, 1)),
            "id128": id128,
            "iota1": iota1,
            "iotaE": iotae,
            "eid": np.full((16, 1), float(c), dtype=np.float32),
            "rep16": rep16,
        })
    return in_maps


def combine_outputs(results):
    """results: list (per core) of {name: np.ndarray}. Returns full outputs."""
    logits = np.zeros((N, O), dtype=np.float32)
    probs = np.zeros((N, E), dtype=np.float32)
    counts = np.zeros((E,), dtype=np.float32)
    for c in range(NC):
        r = results[c]
        probs[c * NLOC:(c + 1) * NLOC] = np.asarray(r["probs"]).reshape(NLOC, E)
        m = int(np.asarray(r["cnt"]).reshape(-1)[0])  # includes sentinel
        counts[c] = m - 1
        mu = min(m, CAP)                              # rows actually gathered
        ids = np.asarray(r["idx"]).T.ravel()[:mu].astype(np.int64)
        yrows = np.asarray(r["y"]).reshape(CAP, O)[:mu]
        logits[ids[1:]] = yrows[1:]
    return logits, probs, counts


def kernel(x, Wr, br, W1, b1, W2, b2):
    x = np.ascontiguousarray(np.asarray(x, dtype=np.float32))
    Wr = np.ascontiguousarray(np.asarray(Wr, dtype=np.float32))
    br = np.ascontiguousarray(np.asarray(br, dtype=np.float32))
    W1 = np.ascontiguousarray(np.asarray(W1, dtype=np.float32))
    b1 = np.ascontiguousarray(np.asarray(b1, dtype=np.float32))
    W2 = np.ascontiguousarray(np.asarray(W2, dtype=np.float32))
    b2 = np.ascontiguousarray(np.asarray(b2, dtype=np.float32))

    nc = build_program()
    in_maps = make_in_maps(x, Wr, br, W1, b1, W2, b2)
    res = bass_utils.run_bass_kernel_spmd(nc, in_maps, core_ids=list(range(NC)))
    return combine_outputs(res.results)
